# revision 1
# baseline (speedup 1.0000x reference)
"""Trainium2 Bass kernel for nn_NetBA_9466107920964 (GIN message passing).

Strategy: 8-way contiguous node sharding (6250 nodes/core padded to 6272).
Edges (+ self-loops) are bucketed by dst core, 128-node dst block, and src
quarter; aggregation gathers source rows with dma_gather (fp8 tables) and
accumulates per-dst-block via one-hot matmuls in PSUM. The per-layer linear
runs feature-major so BatchNorm batch statistics use ACT accum_out columns;
stats are AllReduced (the only per-feature collective), activations are
AllGathered quarter-by-quarter as fp8 gather tables for the next layer.
Mean-pool partial sums per graph are computed with one-hot matmuls and
combined on the host.
"""

import numpy as np
import ml_dtypes

import concourse.bass as bass
import concourse.mybir as mybir
import concourse.tile as tile
from concourse import bacc
from concourse import bass_utils

F8NP = ml_dtypes.float8_e4m3fn
BFNP = ml_dtypes.bfloat16
N, E, F_IN, DIM, HID, G = 50000, 800000, 64, 256, 128, 64
NCORES = 8
NP_OWN = 6250
NBLK = 49
NP_PAD = NBLK * 128            # 6272
NPAD_TOT = NCORES * NP_PAD     # 50176
BN_EPS = 1e-5
CALLCH = 32
QB = [(0, 12), (12, 24), (24, 36), (36, 49)]   # block ranges per quarter
QROWS = [(b1 - b0) * 128 for b0, b1 in QB]     # rows per core per quarter
QTROWS = [NCORES * r for r in QROWS]           # table rows per quarter
F32 = mybir.dt.float32
BF16 = mybir.dt.bfloat16
F8 = mybir.dt.float8e4
I16 = mybir.dt.int16
AF = mybir.ActivationFunctionType
ALU = mybir.AluOpType


def preprocess(edge_index, batch):
    src = np.concatenate([edge_index[0].astype(np.int64), np.arange(N)])
    dst = np.concatenate([edge_index[1].astype(np.int64), np.arange(N)])
    srcp = (src // NP_OWN) * NP_PAD + (src % NP_OWN)
    dstp = (dst // NP_OWN) * NP_PAD + (dst % NP_OWN)
    core = dst // NP_OWN
    blk = (dstp % NP_PAD) // 128
    dloc = dstp % 128
    sb = (srcp % NP_PAD) // 128
    q = np.minimum(sb // 12, 3)
    # gather index within quarter table q
    sc = srcp // NP_PAD
    qstart = np.array([b0 * 128 for b0, _ in QB])
    qsz = np.array(QROWS)
    gidx = sc * qsz[q] + (srcp % NP_PAD) - qstart[q]

    key = (core * NBLK + blk) * 4 + q
    cnt = np.bincount(key, minlength=NCORES * NBLK * 4).reshape(NCORES, NBLK, 4)
    nch_bq = np.maximum((cnt.max(axis=0) + 127) // 128, 1)   # [NBLK, 4]

    chunk_block, chunk_first, chunk_last = [], [], []
    calls = []   # (round, chunk_start, nch)
    off = 0
    for rnd in range(4):
        rb = []
        for b in range(NBLK):
            n = int(nch_bq[b, rnd])
            for j in range(n):
                rb.append(b)
                chunk_first.append(j == 0)
                chunk_last.append(j == n - 1)
                chunk_block.append(b)
        s = 0
        while s < len(rb):
            take = min(CALLCH, len(rb) - s)
            calls.append((rnd, off + s, take))
            s += take
        off += len(rb)
    nch_tot = len(chunk_block)

    order = np.lexsort((q, blk, core))
    gidx_s, dloc_s = gidx[order], dloc[order]
    starts = np.zeros((NCORES, NBLK, 4), np.int64)
    pos = 0
    for c in range(NCORES):
        for b in range(NBLK):
            for h in range(4):
                starts[c, b, h] = pos
                pos += cnt[c, b, h]

    idx16 = np.zeros((NCORES, 128, nch_tot * 8), np.int16)
    oh = np.zeros((NCORES, 128, nch_tot * 128), F8NP)
    for c in range(NCORES):
        vals = np.zeros(nch_tot * 128, np.int64)
        dl = np.full(nch_tot * 128, 128, np.int64)
        ci = 0
        for rnd in range(4):
            for b in range(NBLK):
                n = int(nch_bq[b, rnd])
                e0 = starts[c, b, rnd]
                k = cnt[c, b, rnd]
                sl = slice(ci * 128, ci * 128 + k)
                vals[sl] = gidx_s[e0:e0 + k]
                dl[sl] = dloc_s[e0:e0 + k]
                ci += n
        assert ci == nch_tot
        w = vals.astype(np.int16).reshape(nch_tot * 8, 16).T
        idx16[c] = np.tile(w, (8, 1))
        i_all = np.arange(nch_tot * 128)
        m = dl < 128
        oh[c][i_all[m] % 128, (i_all[m] // 128) * 128 + dl[m]] = 1.0

    batch = np.asarray(batch).astype(np.int64)
    Gb = np.zeros((NCORES, 128, NBLK * G), np.float32)
    for c in range(NCORES):
        gi = batch[c * NP_OWN:(c + 1) * NP_OWN]
        loc = np.arange(NP_OWN)
        Gb[c, loc % 128, (loc // 128) * G + gi] = 1.0
    counts = np.bincount(batch, minlength=G).astype(np.float32)

    meta = dict(nch_tot=nch_tot, calls=calls,
                chunk_block=np.array(chunk_block),
                chunk_first=np.array(chunk_first),
                chunk_last=np.array(chunk_last))
    return meta, idx16, oh, Gb, counts


def build_program(meta, scalars, repeat=1, skip_ar=False, skip_ag=False,
                  skip_gather=False):
    nch_tot = meta["nch_tot"]
    calls = meta["calls"]
    cblock = meta["chunk_block"]
    cfirst = meta["chunk_first"]
    clast = meta["chunk_last"]

    nc = bacc.Bacc("TRN2", target_bir_lowering=False, debug=False,
                   enable_asserts=False, num_devices=NCORES)
    dt = nc.dram_tensor
    xq = [dt(f"xq{q}", [QTROWS[q], 256], F8, kind="ExternalInput").ap()
          for q in range(4)]
    idx16 = dt("idx16", [128, nch_tot * 8], I16, kind="ExternalInput").ap()
    oh_dram = dt("oh", [128, nch_tot * 128], F8, kind="ExternalInput").ap()
    W1 = dt("W1", [F_IN, DIM], F32, kind="ExternalInput").ap()
    W2 = dt("W2", [DIM, DIM], F32, kind="ExternalInput").ap()
    W3 = dt("W3", [DIM, DIM], F32, kind="ExternalInput").ap()
    Wl1 = dt("Wl1", [DIM, HID], F32, kind="ExternalInput").ap()
    Wl2 = dt("Wl2", [HID, 1], F32, kind="ExternalInput").ap()
    gbb = dt("gbb", [128, 18], F32, kind="ExternalInput").ap()
    bl1c = dt("bl1c", [HID, 1], F32, kind="ExternalInput").ap()
    Gb = dt("Gb", [128, NBLK * G], F32, kind="ExternalInput").ap()
    pool_out = dt("pool_out", [G, 1], F32, kind="ExternalOutput").ap()
    bl2val = float(scalars["bl2"])

    with tile.TileContext(nc) as tc:
        import contextlib
        with contextlib.ExitStack() as ctx:
            const = ctx.enter_context(tc.tile_pool(name="const", bufs=1))
            xep = ctx.enter_context(tc.tile_pool(name="xep", bufs=3))
            ohp = ctx.enter_context(tc.tile_pool(name="ohp", bufs=3))
            aggp = ctx.enter_context(tc.tile_pool(name="aggp", bufs=100))
            sp = ctx.enter_context(tc.tile_pool(name="sp", bufs=4))
            hp = ctx.enter_context(tc.tile_pool(name="hp", bufs=100))
            hsqp = ctx.enter_context(tc.tile_pool(name="hsqp", bufs=2))
            actp = ctx.enter_context(tc.tile_pool(name="actp", bufs=4))
            tiny = ctx.enter_context(tc.tile_pool(name="tiny", bufs=2))
            dram = ctx.enter_context(tc.tile_pool(name="dram", bufs=1, space="DRAM"))
            psb = ctx.enter_context(tc.tile_pool(name="psb", bufs=4, space="PSUM"))
            psh = ctx.enter_context(tc.tile_pool(name="psh", bufs=4, space="PSUM"))

            idx_t = const.tile([128, nch_tot * 8], I16)
            nc.sync.dma_start(idx_t[:], idx16[:])
            W1_t = const.tile([F_IN, DIM], F32)
            nc.sync.dma_start(W1_t[:], W1[:])
            W_t = {1: (W1_t,)}
            for li, W in ((2, W2), (3, W3)):
                hi = const.tile([128, DIM], F32, tag=f"W{li}hi")
                lo = const.tile([128, DIM], F32, tag=f"W{li}lo")
                nc.sync.dma_start(hi[:], W[0:128, :])
                nc.sync.dma_start(lo[:], W[128:256, :])
                W_t[li] = (hi, lo)
            Wl1_hi = const.tile([128, HID], F32, tag="Wl1hi")
            Wl1_lo = const.tile([128, HID], F32, tag="Wl1lo")
            nc.sync.dma_start(Wl1_hi[:], Wl1[0:128, :])
            nc.sync.dma_start(Wl1_lo[:], Wl1[128:256, :])
            Wl2_t = const.tile([HID, 1], F32)
            nc.sync.dma_start(Wl2_t[:], Wl2[:])
            bl1_t = const.tile([HID, 1], F32)
            nc.sync.dma_start(bl1_t[:], bl1c[:])
            Gb_t = const.tile([128, NBLK * G], F32)
            nc.sync.dma_start(Gb_t[:], Gb[:])
            gbb_t = const.tile([128, 18], F32)
            nc.sync.dma_start(gbb_t[:], gbb[:])
            bl2_t = const.tile([128, 1], F32, tag="bl2_t")
            nc.vector.memset(bl2_t[:], bl2val)
            from concourse.masks import make_identity
            ident = const.tile([128, 128], F32, tag="ident")
            make_identity(nc, ident[:])

            for _rep in range(repeat):
                tabs = {}
                ag_in = {}
                for li in (1, 2):
                    tabs[li] = [dram.tile([QTROWS[q], 256], F8,
                                          tag=f"tab{li}{q}{_rep}", name=f"tab{li}{q}",
                                          addr_space=("Local" if skip_ag else "Shared"))
                                for q in range(4)]
                    ag_in[li] = [dram.tile([QROWS[q], 256], F8,
                                           tag=f"agin{li}{q}{_rep}", name=f"agin{li}{q}")
                                 for q in range(4)]
                st_in = {li: dram.tile([128, 4], F32, tag=f"sti{li}{_rep}", name=f"sti{li}")
                         for li in (1, 2, 3)}
                st_out = {li: dram.tile([128, 4], F32, tag=f"sto{li}{_rep}", name=f"sto{li}",
                                        addr_space="Shared")
                          for li in (1, 2, 3)}
                if skip_ag:
                    z8 = actp.tile([128, 256], F8, tag="out8", name="z8")
                    nc.vector.memset(z8[:], 0)
                    for li in (1, 2):
                        for qq in range(4):
                            nc.sync.dma_start(
                                tabs[li][qq][:].rearrange("(a p) d -> p a d", p=128),
                                z8[:, None, 0:256].to_broadcast(
                                    [128, QTROWS[qq] // 128, 256]))

                h3T = [None] * (2 * NBLK)
                for li in (1, 2, 3):
                    srcq = xq if li == 1 else [t[:] for t in tabs[li - 1]]
                    ps_cur = {}
                    aggA = {}
                    h_t = [None] * (2 * NBLK)
                    stat_col = tiny.tile([128, 4], F32, tag="stat_col", name="stat_col")
                    kdone = [0]

                    for rnd, cstart, ncall in calls:
                        xe = xep.tile([128, ncall, 256], F8, tag="xe", name="xe")
                        if skip_gather:
                            nc.gpsimd.memset(xe[:], 0)
                        else:
                            nc.gpsimd.dma_gather(
                                out_ap=xe[:], in_ap=srcq[rnd],
                                idxs_ap=idx_t[:, cstart * 8:(cstart + ncall) * 8],
                                num_idxs=ncall * 128, num_idxs_reg=ncall * 128,
                                elem_size=256, single_packet=False)
                        oh_t = ohp.tile([128, ncall * 128], F8, tag="oh", name="oh_t")
                        nc.sync.dma_start(
                            oh_t[:], oh_dram[:, cstart * 128:(cstart + ncall) * 128])
                        for j in range(ncall):
                            ch = cstart + j
                            b = int(cblock[ch])
                            first, last = bool(cfirst[ch]), bool(clast[ch])
                            if first:
                                if li == 1:
                                    ps_cur[b] = (psb.tile([F_IN, 128], F32,
                                                          tag="psb", name="psA"),)
                                else:
                                    ps_cur[b] = (psb.tile([128, 128], F32,
                                                          tag="psb", name="psA"),
                                                 psb.tile([128, 128], F32,
                                                          tag="psb", name="psB"))
                            pcur = ps_cur[b]
                            rhs = oh_t[:, j * 128:(j + 1) * 128]
                            if li == 1:
                                nc.tensor.matmul(out=pcur[0][:], lhsT=xe[:, j, 0:F_IN],
                                                 rhs=rhs, start=first, stop=last)
                            else:
                                nc.tensor.matmul(out=pcur[0][:], lhsT=xe[:, j, 0:128],
                                                 rhs=rhs, start=first, stop=last)
                                nc.tensor.matmul(out=pcur[1][:], lhsT=xe[:, j, 128:256],
                                                 rhs=rhs, start=first, stop=last)
                            if not last:
                                continue
                            del ps_cur[b]
                            if rnd == 0:
                                tiles = []
                                for t in pcur:
                                    a = aggp.tile(list(t.shape), BF16, tag="aggA",
                                                  name="aggA")
                                    nc.vector.tensor_copy(a[:], t[:])
                                    tiles.append(a)
                                aggA[b] = tuple(tiles)
                                continue
                            if rnd < 3:
                                for t, a in zip(pcur, aggA[b]):
                                    nc.vector.tensor_tensor(out=a[:], in0=t[:],
                                                            in1=a[:], op=ALU.add)
                                continue
                            # rnd == 3: combine + linear + stats
                            s_tiles = []
                            for t, a in zip(pcur, aggA[b]):
                                s = sp.tile(list(t.shape), F32, tag="s", name="s")
                                nc.vector.tensor_tensor(out=s[:], in0=t[:], in1=a[:],
                                                        op=ALU.add)
                                s_tiles.append(s)
                            hps = (psh.tile([128, 128], F32, tag="hT", name="hTa"),
                                   psh.tile([128, 128], F32, tag="hT", name="hTb"))
                            for j2 in range(2):
                                wsl = (slice(0, 128), slice(128, 256))[j2]
                                if li == 1:
                                    nc.tensor.matmul(out=hps[j2][:],
                                                     lhsT=W1_t[:, wsl],
                                                     rhs=s_tiles[0][:],
                                                     start=True, stop=True)
                                else:
                                    nc.tensor.matmul(out=hps[j2][:],
                                                     lhsT=W_t[li][0][:, wsl],
                                                     rhs=s_tiles[0][:],
                                                     start=True, stop=False)
                                    nc.tensor.matmul(out=hps[j2][:],
                                                     lhsT=W_t[li][1][:, wsl],
                                                     rhs=s_tiles[1][:],
                                                     start=False, stop=True)
                            parts = actp.tile([128, 4], F32, tag="parts", name="parts")
                            for j2 in range(2):
                                h_sb = hp.tile([128, 128], F32, tag="hT_sb",
                                               name="h_sb")
                                nc.scalar.activation(h_sb[:], hps[j2][:], AF.Identity,
                                                     accum_out=parts[:, j2:j2 + 1])
                                junk = hsqp.tile([128, 128], BF16, tag="junk",
                                                 name="junk")
                                nc.scalar.activation(junk[:], hps[j2][:], AF.Square,
                                                     accum_out=parts[:, 2 + j2:3 + j2])
                                h_t[2 * b + j2] = h_sb
                            if kdone[0] == 0:
                                nc.vector.tensor_copy(stat_col[:], parts[:])
                            else:
                                nc.vector.tensor_tensor(out=stat_col[:], in0=stat_col[:],
                                                        in1=parts[:], op=ALU.add)
                            kdone[0] += 1

                    # stats allreduce + affine consts (columns [128, 2])
                    nc.sync.dma_start(st_in[li][:], stat_col[:])
                    if skip_ar:
                        nc.sync.dma_start(st_out[li][:], st_in[li][:])
                    else:
                        nc.gpsimd.collective_compute(
                            "AllReduce", ALU.add, replica_groups=[list(range(NCORES))],
                            ins=[st_in[li].opt()], outs=[st_out[li].opt()])
                    gstat = tiny.tile([128, 4], F32, tag="gstat", name="gstat")
                    nc.sync.dma_start(gstat[:], st_out[li][:])
                    o = (li - 1) * 6
                    b_col = gbb_t[:, o:o + 2]
                    g_col = gbb_t[:, o + 2:o + 4]
                    be_col = gbb_t[:, o + 4:o + 6]
                    mean = tiny.tile([128, 2], F32, tag="mean", name="mean")
                    nc.vector.tensor_scalar_mul(mean[:], gstat[:, 0:2], 1.0 / N)
                    var = tiny.tile([128, 2], F32, tag="var", name="var")
                    nc.vector.tensor_scalar_mul(var[:], gstat[:, 2:4], 1.0 / N)
                    m2 = tiny.tile([128, 2], F32, tag="m2", name="m2")
                    nc.vector.tensor_tensor(out=m2[:], in0=mean[:], in1=mean[:],
                                            op=ALU.mult)
                    nc.vector.tensor_tensor(out=var[:], in0=var[:], in1=m2[:],
                                            op=ALU.subtract)
                    nc.vector.tensor_scalar_add(var[:], var[:], BN_EPS)
                    rec = tiny.tile([128, 2], F32, tag="rec", name="rec")
                    nc.vector.reciprocal(rec[:], var[:])
                    a2 = tiny.tile([128, 2], F32, tag="a2", name="a2")
                    nc.scalar.sqrt(a2[:], rec[:])
                    nc.vector.tensor_tensor(out=a2[:], in0=a2[:], in1=g_col, op=ALU.mult)
                    c2 = tiny.tile([128, 2], F32, tag="c2", name="c2")
                    nc.vector.tensor_tensor(out=c2[:], in0=mean[:], in1=b_col, op=ALU.add)
                    nc.vector.tensor_tensor(out=c2[:], in0=c2[:], in1=a2[:], op=ALU.mult)
                    nc.vector.tensor_tensor(out=c2[:], in0=be_col, in1=c2[:],
                                            op=ALU.subtract)

                    # affine (+relu, transpose, cast, store, AG) per quarter
                    for qi, (b0, b1) in enumerate(QB):
                        for b in range(b0, b1):
                            if li < 3:
                                out8 = actp.tile([128, 256], F8, tag="out8",
                                                 name="out8")
                                for j2 in range(2):
                                    act = actp.tile([128, 128], F32, tag="act",
                                                    name="act")
                                    nc.scalar.activation(act[:], h_t[2 * b + j2][:],
                                                         AF.Relu,
                                                         bias=c2[:, j2:j2 + 1],
                                                         scale=a2[:, j2:j2 + 1])
                                    tr = psb.tile([128, 128], F32, tag="psb",
                                                  name="tr")
                                    nc.tensor.transpose(out=tr[:], in_=act[:],
                                                        identity=ident[:])
                                    nc.vector.tensor_copy(
                                        out8[:, j2 * 128:(j2 + 1) * 128], tr[:])
                                r0 = (b - b0) * 128
                                nc.scalar.dma_start(
                                    ag_in[li][qi][r0:r0 + 128, :], out8[:])
                            else:
                                for j2 in range(2):
                                    nc.scalar.activation(h_t[2 * b + j2][:],
                                                         h_t[2 * b + j2][:],
                                                         AF.Identity,
                                                         bias=c2[:, j2:j2 + 1],
                                                         scale=a2[:, j2:j2 + 1])
                                    h3T[2 * b + j2] = h_t[2 * b + j2]
                        if li < 3:
                            if skip_ag:
                                nc.sync.dma_start(
                                    tabs[li][qi][0:QROWS[qi], :], ag_in[li][qi][:])
                            else:
                                nc.gpsimd.collective_compute(
                                    "AllGather", ALU.bypass,
                                    replica_groups=[list(range(NCORES))],
                                    ins=[ag_in[li][qi].opt()],
                                    outs=[tabs[li][qi].opt()])

                # MLP head + pool
                pool_ps = psh.tile([G, 1], F32, tag="hT", name="pool_ps")
                for b in range(NBLK):
                    z1 = psb.tile([HID, 128], F32, tag="psb", name="z1")
                    nc.tensor.matmul(out=z1[:], lhsT=Wl1_hi[:], rhs=h3T[2 * b][:],
                                     start=True, stop=False)
                    nc.tensor.matmul(out=z1[:], lhsT=Wl1_lo[:], rhs=h3T[2 * b + 1][:],
                                     start=False, stop=True)
                    z1s = sp.tile([HID, 128], F32, tag="s", name="z1s")
                    nc.scalar.activation(z1s[:], z1[:], AF.Relu, bias=bl1_t[:, 0:1],
                                         scale=1.0)
                    yps = psh.tile([128, 1], F32, tag="hT", name="yps")
                    nc.tensor.matmul(out=yps[:], lhsT=z1s[:], rhs=Wl2_t[:],
                                     start=True, stop=True)
                    ysb = tiny.tile([128, 1], F32, tag="ysb", name="ysb")
                    nc.scalar.activation(ysb[:], yps[:], AF.Sigmoid,
                                         bias=bl2_t[:, 0:1], scale=1.0)
                    nc.tensor.matmul(out=pool_ps[:], lhsT=Gb_t[:, b * G:(b + 1) * G],
                                     rhs=ysb[:], start=(b == 0), stop=(b == NBLK - 1))
                pool_sb = tiny.tile([G, 1], F32, tag="pool_sb", name="pool_sb")
                nc.vector.tensor_copy(pool_sb[:], pool_ps[:])
                nc.sync.dma_start(pool_out[:], pool_sb[:])

    nc.compile()
    return nc


_cache = {}


def prepare(inputs, repeat=1, **flags):
    key = (repeat, tuple(sorted(flags.items())))
    if key in _cache:
        return _cache[key]
    x = np.asarray(inputs["x"], np.float32)
    meta, idx16, oh, Gb, counts = preprocess(np.asarray(inputs["edge_index"]),
                                             np.asarray(inputs["batch"]))
    # quarter tables of x, padded to 256 fp8 cols
    xqs = []
    for qi, (b0, b1) in enumerate(QB):
        t = np.zeros((QTROWS[qi], 256), F8NP)
        for c in range(NCORES):
            r0, r1 = b0 * 128, b1 * 128
            rows = np.zeros((QROWS[qi], F_IN), np.float32)
            lo = c * NP_OWN + r0
            hi = min(c * NP_OWN + r1, (c + 1) * NP_OWN)
            if hi > lo:
                rows[0:hi - lo] = x[lo:hi]
            t[c * QROWS[qi]:(c + 1) * QROWS[qi], 0:F_IN] = rows.astype(F8NP)
        xqs.append(t)

    gbb = np.zeros((128, 18), np.float32)
    for li, (bk, gk, bek) in enumerate((("b1", "g1", "be1"), ("b2", "g2", "be2"),
                                        ("b3", "g3", "be3"))):
        o = li * 6
        for k, key2 in enumerate((bk, gk, bek)):
            v = np.asarray(inputs[key2], np.float32)
            gbb[:, o + 2 * k] = v[0:128]
            gbb[:, o + 2 * k + 1] = v[128:256]

    scalars = {"bl2": np.asarray(inputs["bl2"]).ravel()[0]}
    nc = build_program(meta, scalars, repeat=repeat, **flags)

    in_maps = []
    for c in range(NCORES):
        m = {f"xq{q}": xqs[q] for q in range(4)}
        m.update({
            "idx16": idx16[c], "oh": oh[c],
            "W1": np.asarray(inputs["W1"], np.float32),
            "W2": np.asarray(inputs["W2"], np.float32),
            "W3": np.asarray(inputs["W3"], np.float32),
            "Wl1": np.asarray(inputs["Wl1"], np.float32),
            "Wl2": np.asarray(inputs["Wl2"], np.float32),
            "gbb": gbb,
            "bl1c": np.asarray(inputs["bl1"], np.float32).reshape(HID, 1),
            "Gb": Gb[c],
        })
        in_maps.append(m)
    _cache[key] = (nc, in_maps, counts)
    return _cache[key]


def execute(nc, in_maps, counts, trace=False):
    res = bass_utils.run_bass_kernel_spmd(nc, in_maps, core_ids=list(range(NCORES)),
                                          trace=trace)
    pool = sum(r["pool_out"] for r in res.results)
    out = (pool / np.maximum(counts, 1.0)[:, None]).astype(np.float32)
    return out, res


def run(inputs, repeat=1, trace=False, **flags):
    nc, in_maps, counts = prepare(inputs, repeat=repeat, **flags)
    return execute(nc, in_maps, counts, trace=trace)


def kernel(**inputs):
    """Full inputs (as in setup_inputs()) -> full [64, 1] float32 output."""
    out, _res = run(inputs, repeat=1)
    return out

